# revision 10
# baseline (speedup 1.0000x reference)
"""Trainium2 Bass kernel for the Brill-Lindquist Christoffel-symbol grid.

Math: the reference reduces to
    psi  = 1 + sum_n m_n / (2 r_n),   m = softplus(pre)
    h    = psi^4
    G_c  = finite-difference gradient of h along grid axis c (2nd order
           central interior, 1st order one-sided edges, spacing DX)
    W_c  = 0.5 * G_c / h
    Gamma^i_{jk} = delta_ij W_k + delta_ik W_j - delta_jk W_i
so the [96,96,96,3,3,3] output is +-W_c scattered over 27 slots per point.

Sharding: axis 0 (12 planes per core x 8 cores); h is analytic in the
inputs so each core evaluates its slab + 1-tile halo directly. Rows are
packed row = a0*96+a1 (9 tiles of 128 partitions), free dim = a2.

v3 design (output-DMA roofline ~29 us/core + fill):
  - r_n^2 built by a K=4 PE matmul (outer-sum of host-prescaled ab/crow
    quadratics), one Act sqrt for both BHs, one DVE reciprocal, one Pool
    add: psi-1 = 1/s1 + 1/s2 with s_n = r_n/mh_n. hsq=(psi)^2 fp32;
    h stored bf16 only.
  - axis-0/1 FD: 6 bf16 matmuls/tile (fp32 psum), single-bf16 h
    (rel err 2.9e-3, budget 2e-2). z-FD via shifted subtract.
  - scatter fused into the 27-slot interleaved output tile; engine
    balance: DVE 4 products, Pool 2 negated products + z-FD pieces,
    Act hinv + 3 c2 copies.
  - just-in-time h chunks (blocks 0,1,2 single then pairs) interleaved
    with tiles; deep PE warmup; 4 rotating output buffers.
"""

import numpy as np

RES = 96
N_CORES = 8
PLANES = RES // N_CORES        # 12
LROWS = PLANES * RES           # 1152 local rows
NT = LROWS // 128              # 9 local 128-row tiles
EXTNT = NT + 2                 # 11 extended tiles (halo)
NROWS_G = RES * RES            # 9216 global rows
S27 = 27
NOB = 4                        # rotating output buffers

# quad input [4, QW]: cols 0:192 = r^2 rhs (const), then 11 lhsT blocks
Q_RHS = 0
Q_LHS = 192
QW = 192 + EXTNT * 128

# dmat entry order: tile-0's two entries first so a small leading DMA
# unblocks the first tile's matmuls early.
ORDER = [0, 3, 1, 4, 5, 2]
SLOT = {e: i for i, e in enumerate(ORDER)}

# h-phase chunks (ext-block ranges): singles first for fill latency
CHUNKS = [(0, 1), (1, 2), (2, 3), (3, 5), (5, 7), (7, 9), (9, 11)]
TILES_AFTER_CHUNK = {2: [0], 3: [1, 2], 4: [3, 4], 5: [5, 6], 6: [7, 8]}


def _grid_x():
    # Match the reference grid bit-for-bit: jnp.linspace in fp32 on CPU
    # (the reference's softplus cannot compile for the neuron backend, so
    # it necessarily runs on the jax CPU platform).
    import jax
    import jax.numpy as jnp
    MAX_X = 1.0
    DX = np.float32(MAX_X / (RES / 2 - 1))

    def _ls():
        return jnp.linspace(
            DX * (1 - RES / 2), DX * (RES / 2 - 1), RES, dtype=jnp.float32
        )

    try:
        with jax.default_device(jax.devices("cpu")[0]):
            x = np.asarray(_ls())
    except Exception:
        x = np.asarray(_ls())
    return x, float(DX)


def _fd_sources(idx, coeff_c, coeff_e):
    """(offset, coeff) pairs for d/didx with 1st-order one-sided edges."""
    if idx == 0:
        return [(1, coeff_e), (0, -coeff_e)]
    if idx == RES - 1:
        return [(0, coeff_e), (-1, -coeff_e)]
    return [(1, coeff_c), (-1, -coeff_c)]


def _build_dmat(core, DX):
    """[128, 6*3*128] bf16 FD matrices as matmul lhsT ([q, p] = coeff of
    ext-row q in output row p); 0.5 Christoffel factor folded in. All
    values are +-0.25/DX or +-0.5/DX = +-11.75 / +-23.5, exact in bf16.
    Logical entries: 0 g0(t=0), 1 g0(interior), 2 g0(t=8), 3..5 g1(t%3);
    stored in column slots per ORDER."""
    import ml_dtypes
    c0 = 0.5 * (1.0 / (2.0 * np.float64(DX)))
    ce = 0.5 * (1.0 / np.float64(DX))
    out = np.zeros((128, 6 * 3 * 128), np.float64)

    def fill(entry, t, axis):
        slot = SLOT[entry]
        for p in range(128):
            gr = core * LROWS + 128 * t + p
            a = (gr // RES) if axis == 0 else (gr % RES)
            step = RES if axis == 0 else 1
            for off, cf in _fd_sources(a, c0, ce):
                g2 = gr + off * step
                e_ = g2 - core * LROWS + 128
                j = e_ // 128 - t
                q = e_ - 128 * (t + j)
                assert 0 <= j <= 2 and 0 <= q < 128, (core, t, p, off)
                out[q, (slot * 3 + j) * 128 + p] = cf

    fill(0, 0, 0)
    fill(1, 1, 0)
    fill(2, NT - 1, 0)
    for v in range(3):
        fill(3 + v, v, 1)
    return out.astype(ml_dtypes.bfloat16)


def _g0_slot(t):
    return SLOT[0] if t == 0 else (SLOT[2] if t == NT - 1 else SLOT[1])


def _g1_slot(t):
    return SLOT[3 + (t % 3)]


def _build_static(core, x, DX):
    slab = core * LROWS
    e = np.arange(EXTNT * 128)
    g = np.clip(slab - 128 + e, 0, NROWS_G - 1)   # clamp halo overrun (unused rows)
    xcol = x[g % RES].reshape(EXTNT, 128).T.copy()     # X coordinate (a1)
    ycol = x[g // RES].reshape(EXTNT, 128).T.copy()    # Y coordinate (a0)
    kvec = np.full(RES, 0.25 / DX, np.float64)
    kvec[0] = kvec[-1] = 0.5 / DX
    return {
        "xcol": np.ascontiguousarray(xcol, np.float64),
        "ycol": np.ascontiguousarray(ycol, np.float64),
        "kv": np.ascontiguousarray(
            np.broadcast_to(kvec[None, :], (128, RES)), np.float32
        ),
        "dmat": _build_dmat(core, DX),
    }


def _build_quad(static, x, pos, mh):
    """[4, QW] fp32: r^2 outer-sum matmul operands, prescaled by 1/mh^2.
    psum[p, n*96+z] = lhsT[:, p] . rhs[:, n*96+z] = ab_n[p] + crow_n[z]."""
    quad = np.zeros((4, QW), np.float64)
    for n in range(2):
        crow = (x.astype(np.float64) - pos[n, 2]) ** 2 / (mh[n] * mh[n])
        quad[2 * n, Q_RHS + RES * n:Q_RHS + RES * (n + 1)] = 1.0
        quad[2 * n + 1, Q_RHS + RES * n:Q_RHS + RES * (n + 1)] = crow
        ab = ((static["xcol"] - pos[n, 0]) ** 2
              + (static["ycol"] - pos[n, 1]) ** 2) / (mh[n] * mh[n])
        # lhsT block b: row 2n = ab_n[:, b], row 2n+1 = 1
        for b in range(EXTNT):
            quad[2 * n, Q_LHS + 128 * b:Q_LHS + 128 * (b + 1)] = ab[:, b]
            quad[2 * n + 1, Q_LHS + 128 * b:Q_LHS + 128 * (b + 1)] = 1.0
    return np.ascontiguousarray(quad, np.float32)


def _build_program():
    import dataclasses as _dc

    import concourse.bacc as bacc
    import concourse.mybir as mybir
    import concourse.tile as tile

    DT = mybir.dt.float32
    BF = mybir.dt.bfloat16
    AF = mybir.ActivationFunctionType

    nc = bacc.Bacc(None, target_bir_lowering=False, debug=True)
    d_quad = nc.dram_tensor("quad", [4, QW], DT, kind="ExternalInput")
    d_kv = nc.dram_tensor("kv", [128, RES], DT, kind="ExternalInput")
    d_dmat = nc.dram_tensor("dmat", [128, 6 * 3 * 128], BF, kind="ExternalInput")
    d_out = nc.dram_tensor("out", [LROWS, RES * S27], DT, kind="ExternalOutput")

    HW_ = EXTNT * RES             # 1056: free width of the ext h field
    with tile.TileContext(nc) as tc:
        with (
            tc.tile_pool(name="const", bufs=1) as cpool,
            tc.tile_pool(name="work", bufs=3) as wpool,
            tc.tile_pool(name="wout", bufs=3) as wopool,
            tc.tile_pool(name="obuf", bufs=NOB) as opool,
            tc.tile_pool(name="psum", bufs=2, space="PSUM") as pspool,
            tc.tile_pool(name="psr2", bufs=2, space="PSUM") as r2pool,
            tc.tile_pool(name="psw", bufs=1, space="PSUM") as pswpool,
        ):
            # --- inputs in: quad first (unblocks r^2 matmuls + h chain),
            # dmat in two pieces (tile-0's entries lead), kvec ---
            qu = cpool.tile([4, QW], DT)
            nc.sync.dma_start(qu[:], d_quad[:])
            dm = cpool.tile([128, 6 * 3 * 128], BF)
            nc.sync.dma_start(dm[:, :6 * 128], d_dmat[:, :6 * 128])
            kv = cpool.tile([128, RES], DT)
            nc.sync.dma_start(kv[:], d_kv[:])
            nc.sync.dma_start(dm[:, 6 * 128:], d_dmat[:, 6 * 128:])

            # --- warmup tiles: Act table preload (sqrt set) now; the PE
            # pstate-ramp matmuls are emitted after chunk 2 (below) so the
            # latency-critical r^2 matmuls aren't queued behind them ---
            junk = cpool.tile([128, 384], BF, tag="junk")
            nc.vector.memset(junk[:], 1.0)
            jact = cpool.tile([1, 8], DT, tag="jact")
            nc.scalar.activation(jact[:], junk[0:1, 0:8], AF.Sqrt)
            jps = pswpool.tile([128, 384], DT)

            # --- persistent fields ---
            HSQ = cpool.tile([128, HW_], DT)   # psi^2, fp32 (hinv source)
            Hb = cpool.tile([128, HW_], BF)    # h = psi^4, bf16 (FD source)
            HINV = cpool.tile([128, HW_], DT)  # 1/h
            HZ = cpool.tile([128, HW_], DT)    # kvec/h (z-FD scale)

            # --- rotating output buffers, zero slots pre-filled once ---
            otiles = []
            for i in range(NOB):
                O = opool.tile([128, RES * S27], DT, tag=f"ob{i}")
                O3 = O[:].rearrange("p (z s) -> p z s", s=S27)
                eng = nc.vector if i < 2 else nc.gpsimd
                eng.memset(O3[:, :, 5:8:2], 0.0)
                eng.memset(O3[:, :, 11:20:4], 0.0)
                eng.memset(O3[:, :, 21], 0.0)
                otiles.append(O)

            def h_chunk(b0, b1):
                nb = b1 - b0
                W = nb * RES
                csl = slice(RES * b0, RES * b1)
                # r_n^2/mh_n^2 for both BHs: one K=4 matmul per block
                ps = r2pool.tile([128, 2 * W], DT, tag="r2")
                for k, b in enumerate(range(b0, b1)):
                    nc.tensor.matmul(
                        ps[:, 192 * k:192 * (k + 1)],
                        qu[:, Q_LHS + 128 * b:Q_LHS + 128 * (b + 1)],
                        qu[:, Q_RHS:Q_RHS + 192],
                        start=True, stop=True,
                    )
                s = wpool.tile([128, 2 * W], DT, tag="s")
                nc.scalar.activation(s[:], ps[:], AF.Sqrt)
                sinv = wpool.tile([128, 2 * W], DT, tag="sinv")
                nc.vector.reciprocal_approx_fast(sinv[:], s[:])
                psim = wpool.tile([128, W], DT, tag="psim")
                pv = psim[:].rearrange("p (b z) -> p b z", z=RES)
                sv = sinv[:].rearrange("p (b z) -> p b z", z=RES)
                nc.gpsimd.tensor_add(pv[:, :, :], sv[:, 0:2 * nb:2, :], sv[:, 1:2 * nb:2, :])
                nc.scalar.activation(HSQ[:, csl], psim[:], AF.Square, bias=1.0)
                nc.scalar.activation(Hb[:, csl], HSQ[:, csl], AF.Square)
                qc = wpool.tile([128, W], DT, tag="qc")
                nc.vector.reciprocal_approx_fast(qc[:], HSQ[:, csl])
                nc.scalar.activation(HINV[:, csl], qc[:], AF.Square)
                hzv = HZ[:, csl].rearrange("p (b z) -> p b z", z=RES)
                kvb = _dc.replace(kv[:], ap=[kv[:].ap[0], [0, nb], [1, RES]])
                hiv = HINV[:, csl].rearrange("p (b z) -> p b z", z=RES)
                nc.gpsimd.tensor_mul(hzv[:, :, :], hiv[:, :, :], kvb)

            def do_tile(t):
                hsl = slice(RES * (t + 1), RES * (t + 2))
                p0 = pspool.tile([128, RES], DT, tag="p0")
                p1 = pspool.tile([128, RES], DT, tag="p1")
                for slot, pp in ((_g0_slot(t), p0), (_g1_slot(t), p1)):
                    for j in range(3):
                        lhs = dm[:, (slot * 3 + j) * 128:(slot * 3 + j + 1) * 128]
                        rsl = slice(RES * (t + j), RES * (t + j + 1))
                        nc.tensor.matmul(
                            pp[:], lhs, Hb[:, rsl], start=(j == 0), stop=(j == 2)
                        )

                hi = HINV[:, hsl]
                st = wopool.tile([128, RES], DT, tag="st")
                Ht = Hb[:, hsl]
                nc.vector.tensor_sub(st[:, 1:95], Ht[:, 2:96], Ht[:, 0:94])
                nc.gpsimd.tensor_sub(st[:, 0:1], Ht[:, 1:2], Ht[:, 0:1])
                nc.gpsimd.tensor_sub(st[:, 95:96], Ht[:, 95:96], Ht[:, 94:95])
                w2 = wopool.tile([128, RES], DT, tag="w2")
                nc.gpsimd.tensor_mul(w2[:], st[:], HZ[:, hsl])
                # W0/W1 from psum (DVE only: GPSIMD can't read PSUM)
                v0 = wopool.tile([128, RES], DT, tag="v0")
                nc.vector.tensor_mul(v0[:], p0[:], hi)
                v1 = wopool.tile([128, RES], DT, tag="v1")
                nc.vector.tensor_mul(v1[:], p1[:], hi)

                O = otiles[t % NOB]
                O3 = O[:].rearrange("p (z s) -> p z s", s=S27)

                def bcast(ap_, k):
                    return _dc.replace(ap_, ap=ap_.ap + [[0, k]])

                # c=0: +W0 at {0,12,24},{10,20}; -W0 at {4,8}
                nc.vector.tensor_copy(O3[:, :, 0:25:12], bcast(v0[:], 3))
                nc.vector.tensor_copy(O3[:, :, 10:21:10], bcast(v0[:], 2))
                nc.gpsimd.tensor_scalar_mul(O3[:, :, 4:9:4], bcast(v0[:], 2), -1.0)
                # c=1: +W1 at {1,13,25},{3,23}; -W1 at {9,17}
                nc.vector.tensor_copy(O3[:, :, 1:26:12], bcast(v1[:], 3))
                nc.gpsimd.tensor_copy(O3[:, :, 3:24:20], bcast(v1[:], 2))
                nc.gpsimd.tensor_scalar_mul(O3[:, :, 9:18:8], bcast(v1[:], 2), -1.0)
                # c=2: +w2 at {2,14,26},{6,16}; -w2 at {18,22}
                nc.scalar.copy(O3[:, :, 2:27:12], bcast(w2[:], 3))
                nc.scalar.copy(O3[:, :, 6:17:10], bcast(w2[:], 2))
                nc.scalar.mul(O3[:, :, 18:23:4], bcast(w2[:], 2), -1.0)

                nc.sync.dma_start(d_out[128 * t:128 * (t + 1), :], O[:])

            for ci, (b0, b1) in enumerate(CHUNKS):
                h_chunk(b0, b1)
                if ci == 2:
                    # PE pstate ramp while the h chain races ahead; ends
                    # roughly when tile 0's FD matmuls become runnable
                    for _ in range(10):
                        nc.tensor.matmul(
                            jps[:], junk[:, :128], junk[:], start=True, stop=True
                        )
                for t in TILES_AFTER_CHUNK.get(ci, []):
                    do_tile(t)

    nc.finalize()
    return nc


_CACHE = {}


def _get_setup():
    if "nc" not in _CACHE:
        x, DX = _grid_x()
        _CACHE["x"] = x
        _CACHE["static"] = [_build_static(c, x, DX) for c in range(N_CORES)]
        _CACHE["nc"] = _build_program()
    return _CACHE["nc"], _CACHE["static"]


def _in_maps(BH_positions, BH_masses_presoftplus):
    nc, static = _get_setup()
    x = _CACHE["x"]
    pos = np.asarray(BH_positions, np.float64)
    pre = np.asarray(BH_masses_presoftplus, np.float64)
    mh = np.log1p(np.exp(pre)) * 0.5          # softplus(pre) / 2
    return [
        {
            "quad": _build_quad(static[c], x, pos, mh),
            "kv": static[c]["kv"],
            "dmat": static[c]["dmat"],
        }
        for c in range(N_CORES)
    ]


def kernel(BH_positions, BH_masses_presoftplus):
    from concourse.bass_utils import run_bass_kernel_spmd

    nc, _ = _get_setup()
    in_maps = _in_maps(BH_positions, BH_masses_presoftplus)
    res = run_bass_kernel_spmd(nc, in_maps, list(range(N_CORES)))
    parts = [
        res.results[c]["out"].reshape(PLANES, RES, RES, 3, 3, 3)
        for c in range(N_CORES)
    ]
    return np.ascontiguousarray(np.concatenate(parts, axis=0))


# revision 13
# speedup vs baseline: 1.3182x; 1.3182x over previous
"""Trainium2 Bass kernel for the Brill-Lindquist Christoffel-symbol grid.

Math: the reference reduces to
    psi  = 1 + sum_n m_n / (2 r_n),   m = softplus(pre)
    h    = psi^4
    G_c  = finite-difference gradient of h along grid axis c (2nd order
           central interior, 1st order one-sided edges, spacing DX)
    W_c  = 0.5 * G_c / h
    Gamma^i_{jk} = delta_ij W_k + delta_ik W_j - delta_jk W_i
so the [96,96,96,3,3,3] output is +-W_c scattered over 27 slots per point.

Sharding: axis 0 (12 planes per core x 8 cores); h is analytic in the
inputs so each core evaluates its slab + 1-tile halo directly. Rows are
packed row = a0*96+a1 (9 tiles of 128 partitions), free dim = a2.

v3 design (output-DMA roofline ~29 us/core + fill):
  - r_n^2 built by a K=4 PE matmul (outer-sum of host-prescaled ab/crow
    quadratics), one Act sqrt for both BHs, one DVE reciprocal, one Pool
    add: psi-1 = 1/s1 + 1/s2 with s_n = r_n/mh_n. hsq=(psi)^2 fp32;
    h stored bf16 only.
  - axis-0/1 FD: 6 bf16 matmuls/tile (fp32 psum), single-bf16 h
    (rel err 2.9e-3, budget 2e-2). z-FD via shifted subtract.
  - scatter fused into the 27-slot interleaved output tile; engine
    balance: DVE 4 products, Pool 2 negated products + z-FD pieces,
    Act hinv + 3 c2 copies.
  - just-in-time h chunks (blocks 0,1,2 single then pairs) interleaved
    with tiles; deep PE warmup; 4 rotating output buffers.
"""

import numpy as np

RES = 96
N_CORES = 8
PLANES = RES // N_CORES        # 12
LROWS = PLANES * RES           # 1152 local rows
NT = LROWS // 128              # 9 local 128-row tiles
EXTNT = NT + 2                 # 11 extended tiles (halo)
NROWS_G = RES * RES            # 9216 global rows
S27 = 27
NOB = 4                        # rotating output buffers

# quad input [4, QW]: cols 0:192 = r^2 rhs (const), then 11 lhsT blocks
Q_RHS = 0
Q_LHS = 192
QW = 192 + EXTNT * 128

# dmat entry order: tile-0's two entries first so a small leading DMA
# unblocks the first tile's matmuls early.
ORDER = [0, 3, 1, 4, 5, 2]
SLOT = {e: i for i, e in enumerate(ORDER)}

# h-phase chunks (ext-block ranges): singles first for fill latency
CHUNKS = [(0, 1), (1, 2), (2, 3), (3, 5), (5, 7), (7, 9), (9, 11)]
TILES_AFTER_CHUNK = {2: [0], 3: [1, 2], 4: [3, 4], 5: [5, 6], 6: [7, 8]}


def _grid_x():
    # Match the reference grid bit-for-bit: jnp.linspace in fp32 on CPU
    # (the reference's softplus cannot compile for the neuron backend, so
    # it necessarily runs on the jax CPU platform).
    import jax
    import jax.numpy as jnp
    MAX_X = 1.0
    DX = np.float32(MAX_X / (RES / 2 - 1))

    def _ls():
        return jnp.linspace(
            DX * (1 - RES / 2), DX * (RES / 2 - 1), RES, dtype=jnp.float32
        )

    try:
        with jax.default_device(jax.devices("cpu")[0]):
            x = np.asarray(_ls())
    except Exception:
        x = np.asarray(_ls())
    return x, float(DX)


def _fd_sources(idx, coeff_c, coeff_e):
    """(offset, coeff) pairs for d/didx with 1st-order one-sided edges."""
    if idx == 0:
        return [(1, coeff_e), (0, -coeff_e)]
    if idx == RES - 1:
        return [(0, coeff_e), (-1, -coeff_e)]
    return [(1, coeff_c), (-1, -coeff_c)]


def _build_dmat(core, DX):
    """[128, 6*3*128] bf16 FD matrices as matmul lhsT ([q, p] = coeff of
    ext-row q in output row p); 0.5 Christoffel factor folded in. All
    values are +-0.25/DX or +-0.5/DX = +-11.75 / +-23.5, exact in bf16.
    Logical entries: 0 g0(t=0), 1 g0(interior), 2 g0(t=8), 3..5 g1(t%3);
    stored in column slots per ORDER."""
    import ml_dtypes
    c0 = 0.5 * (1.0 / (2.0 * np.float64(DX)))
    ce = 0.5 * (1.0 / np.float64(DX))
    out = np.zeros((128, 6 * 3 * 128), np.float64)

    def fill(entry, t, axis):
        slot = SLOT[entry]
        for p in range(128):
            gr = core * LROWS + 128 * t + p
            a = (gr // RES) if axis == 0 else (gr % RES)
            step = RES if axis == 0 else 1
            for off, cf in _fd_sources(a, c0, ce):
                g2 = gr + off * step
                e_ = g2 - core * LROWS + 128
                j = e_ // 128 - t
                q = e_ - 128 * (t + j)
                assert 0 <= j <= 2 and 0 <= q < 128, (core, t, p, off)
                out[q, (slot * 3 + j) * 128 + p] = cf

    fill(0, 0, 0)
    fill(1, 1, 0)
    fill(2, NT - 1, 0)
    for v in range(3):
        fill(3 + v, v, 1)
    return out.astype(ml_dtypes.bfloat16)


def _g0_slot(t):
    return SLOT[0] if t == 0 else (SLOT[2] if t == NT - 1 else SLOT[1])


def _g1_slot(t):
    return SLOT[3 + (t % 3)]


def _build_static(core, x, DX):
    slab = core * LROWS
    e = np.arange(EXTNT * 128)
    g = np.clip(slab - 128 + e, 0, NROWS_G - 1)   # clamp halo overrun (unused rows)
    xcol = x[g % RES].reshape(EXTNT, 128).T.copy()     # X coordinate (a1)
    ycol = x[g // RES].reshape(EXTNT, 128).T.copy()    # Y coordinate (a0)
    kvec = np.full(RES, 0.25 / DX, np.float64)
    kvec[0] = kvec[-1] = 0.5 / DX
    return {
        "xcol": np.ascontiguousarray(xcol, np.float64),
        "ycol": np.ascontiguousarray(ycol, np.float64),
        "kv": np.ascontiguousarray(
            np.broadcast_to(kvec[None, :], (128, RES)), np.float32
        ),
        "dmat": _build_dmat(core, DX),
    }


def _build_quad(static, x, pos, mh):
    """[4, QW] fp32: r^2 outer-sum matmul operands, prescaled by 1/mh^2.
    psum[p, n*96+z] = lhsT[:, p] . rhs[:, n*96+z] = ab_n[p] + crow_n[z]."""
    quad = np.zeros((4, QW), np.float64)
    for n in range(2):
        crow = (x.astype(np.float64) - pos[n, 2]) ** 2 / (mh[n] * mh[n])
        quad[2 * n, Q_RHS + RES * n:Q_RHS + RES * (n + 1)] = 1.0
        quad[2 * n + 1, Q_RHS + RES * n:Q_RHS + RES * (n + 1)] = crow
        ab = ((static["xcol"] - pos[n, 0]) ** 2
              + (static["ycol"] - pos[n, 1]) ** 2) / (mh[n] * mh[n])
        # lhsT block b: row 2n = ab_n[:, b], row 2n+1 = 1
        for b in range(EXTNT):
            quad[2 * n, Q_LHS + 128 * b:Q_LHS + 128 * (b + 1)] = ab[:, b]
            quad[2 * n + 1, Q_LHS + 128 * b:Q_LHS + 128 * (b + 1)] = 1.0
    return np.ascontiguousarray(quad, np.float32)


def _build_program():
    import dataclasses as _dc

    import concourse.bacc as bacc
    import concourse.mybir as mybir
    import concourse.tile as tile

    DT = mybir.dt.float32
    BF = mybir.dt.bfloat16
    AF = mybir.ActivationFunctionType

    nc = bacc.Bacc(None, target_bir_lowering=False, debug=True)
    d_quad = nc.dram_tensor("quad", [4, QW], DT, kind="ExternalInput")
    d_kv = nc.dram_tensor("kv", [128, RES], DT, kind="ExternalInput")
    d_dmat = nc.dram_tensor("dmat", [128, 6 * 3 * 128], BF, kind="ExternalInput")
    d_out = nc.dram_tensor("out", [LROWS, RES * S27], DT, kind="ExternalOutput")

    HW_ = EXTNT * RES             # 1056: free width of the ext h field
    with tile.TileContext(nc) as tc:
        with (
            tc.tile_pool(name="const", bufs=1) as cpool,
            tc.tile_pool(name="work", bufs=3) as wpool,
            tc.tile_pool(name="wout", bufs=3) as wopool,
            tc.tile_pool(name="obuf", bufs=NOB) as opool,
            tc.tile_pool(name="psum", bufs=2, space="PSUM") as pspool,
            tc.tile_pool(name="psr2", bufs=2, space="PSUM") as r2pool,
            tc.tile_pool(name="psw", bufs=1, space="PSUM") as pswpool,
        ):
            # --- inputs in: quad first (unblocks r^2 matmuls + h chain),
            # dmat in two pieces (tile-0's entries lead), kvec ---
            qu = cpool.tile([4, QW], DT)
            nc.sync.dma_start(qu[:], d_quad[:])
            dm = cpool.tile([128, 6 * 3 * 128], BF)
            nc.sync.dma_start(dm[:, :6 * 128], d_dmat[:, :6 * 128])
            kv = cpool.tile([128, RES], DT)
            nc.sync.dma_start(kv[:], d_kv[:])
            nc.sync.dma_start(dm[:, 6 * 128:], d_dmat[:, 6 * 128:])

            # --- warmup tiles: Act table preload (sqrt set) now; the PE
            # pstate-ramp matmuls are emitted after chunk 2 (below) so the
            # latency-critical r^2 matmuls aren't queued behind them ---
            junk = cpool.tile([128, 384], BF, tag="junk")
            nc.vector.memset(junk[:], 1.0)
            jact = cpool.tile([1, 8], DT, tag="jact")
            nc.scalar.activation(jact[:], junk[0:1, 0:8], AF.Sqrt)
            jps = pswpool.tile([128, 384], DT)

            # --- persistent fields ---
            HSQ = cpool.tile([128, HW_], DT)   # psi^2, fp32 (hinv source)
            Hb = cpool.tile([128, HW_], BF)    # h = psi^4, bf16 (FD source)
            HINV = cpool.tile([128, HW_], DT)  # 1/h
            HZ = cpool.tile([128, HW_], DT)    # kvec/h (z-FD scale)

            # --- rotating output buffers, zero slots pre-filled once ---
            otiles = []
            for i in range(NOB):
                O = opool.tile([128, RES * S27], DT, tag=f"ob{i}", bufs=1)
                O3 = O[:].rearrange("p (z s) -> p z s", s=S27)
                eng = nc.vector if i < 2 else nc.gpsimd
                eng.memset(O3[:, :, 5:8:2], 0.0)
                eng.memset(O3[:, :, 11:20:4], 0.0)
                eng.memset(O3[:, :, 21], 0.0)
                otiles.append(O)

            def h_chunk(b0, b1):
                nb = b1 - b0
                W = nb * RES
                csl = slice(RES * b0, RES * b1)
                # r_n^2/mh_n^2 for both BHs: one K=4 matmul per block
                ps = r2pool.tile([128, 2 * W], DT, tag="r2")
                for k, b in enumerate(range(b0, b1)):
                    nc.tensor.matmul(
                        ps[:, 192 * k:192 * (k + 1)],
                        qu[:, Q_LHS + 128 * b:Q_LHS + 128 * (b + 1)],
                        qu[:, Q_RHS:Q_RHS + 192],
                        start=True, stop=True,
                    )
                s = wpool.tile([128, 2 * W], DT, tag="s")
                nc.scalar.activation(s[:], ps[:], AF.Sqrt)
                sinv = wpool.tile([128, 2 * W], DT, tag="sinv")
                nc.vector.reciprocal_approx_fast(sinv[:], s[:])
                psim = wpool.tile([128, W], DT, tag="psim")
                pv = psim[:].rearrange("p (b z) -> p b z", z=RES)
                sv = sinv[:].rearrange("p (b z) -> p b z", z=RES)
                nc.gpsimd.tensor_add(pv[:, :, :], sv[:, 0:2 * nb:2, :], sv[:, 1:2 * nb:2, :])
                # hsq = (psim+1)^2 on DVE, h = hsq^2 (bf16) on Pool: keeps
                # Act free for the scatter grids (Act is the only engine
                # that handles strided writes at full rate)
                tp = wpool.tile([128, W], DT, tag="tp")
                nc.vector.tensor_scalar_add(tp[:], psim[:], 1.0)
                nc.vector.tensor_mul(HSQ[:, csl], tp[:], tp[:])
                nc.gpsimd.tensor_mul(Hb[:, csl], HSQ[:, csl], HSQ[:, csl])
                qc = wpool.tile([128, W], DT, tag="qc")
                nc.vector.reciprocal_approx_fast(qc[:], HSQ[:, csl])
                nc.vector.tensor_mul(HINV[:, csl], qc[:], qc[:])
                hzv = HZ[:, csl].rearrange("p (b z) -> p b z", z=RES)
                kvb = _dc.replace(kv[:], ap=[kv[:].ap[0], [0, nb], [1, RES]])
                hiv = HINV[:, csl].rearrange("p (b z) -> p b z", z=RES)
                nc.gpsimd.tensor_mul(hzv[:, :, :], hiv[:, :, :], kvb)

            def do_tile(t):
                hsl = slice(RES * (t + 1), RES * (t + 2))
                p0 = pspool.tile([128, RES], DT, tag="p0")
                p1 = pspool.tile([128, RES], DT, tag="p1")
                for slot, pp in ((_g0_slot(t), p0), (_g1_slot(t), p1)):
                    for j in range(3):
                        lhs = dm[:, (slot * 3 + j) * 128:(slot * 3 + j + 1) * 128]
                        rsl = slice(RES * (t + j), RES * (t + j + 1))
                        nc.tensor.matmul(
                            pp[:], lhs, Hb[:, rsl], start=(j == 0), stop=(j == 2)
                        )

                st = wopool.tile([128, RES], DT, tag="st")
                Ht = Hb[:, hsl]
                nc.gpsimd.tensor_sub(st[:, 1:95], Ht[:, 2:96], Ht[:, 0:94])
                nc.gpsimd.tensor_sub(st[:, 0:1], Ht[:, 1:2], Ht[:, 0:1])
                nc.gpsimd.tensor_sub(st[:, 95:96], Ht[:, 95:96], Ht[:, 94:95])
                # planar W = [W0 | W1 | W2], contiguous 96-col blocks (DVE:
                # p0/p1 live in PSUM which Pool can't read)
                w3 = wopool.tile([128, 3 * RES], DT, tag="w3")
                nc.vector.tensor_mul(w3[:, 0:RES], p0[:], HINV[:, hsl])
                nc.vector.tensor_mul(w3[:, RES:2 * RES], p1[:], HINV[:, hsl])
                nc.vector.tensor_mul(w3[:, 2 * RES:], st[:], HZ[:, hsl])

                O = otiles[t % NOB]
                pdim = O[:].ap[0]
                wp = w3[:].ap[0]

                # Gamma^i_{jk} = d_ij W_k + d_ik W_j - d_jk W_i scattered as
                # three 3x3 affine slot-grids over s = 9i+3j+k, all on Act
                # (the only engine with full-rate strided writes). The -W_i
                # grid (s=9i+4j) spills -W_i onto the i==j diagonal slots
                # {0,13,26}; the two +W grids rewrite them correctly after.
                nc.scalar.mul(
                    _dc.replace(O[:], ap=[pdim, [S27, RES], [9, 3], [4, 3]]),
                    _dc.replace(w3[:], ap=[wp, [1, RES], [RES, 3], [0, 3]]),
                    -1.0,
                )
                # s = 12i+c (diag i==j rows): +W_c
                nc.scalar.copy(
                    _dc.replace(O[:], ap=[pdim, [S27, RES], [12, 3], [1, 3]]),
                    _dc.replace(w3[:], ap=[wp, [1, RES], [0, 3], [RES, 3]]),
                )
                # s = 10i+3j (i==k plane incl diagonal): +W_j
                nc.scalar.copy(
                    _dc.replace(O[:], ap=[pdim, [S27, RES], [10, 3], [3, 3]]),
                    _dc.replace(w3[:], ap=[wp, [1, RES], [0, 3], [RES, 3]]),
                )

                nc.sync.dma_start(d_out[128 * t:128 * (t + 1), :], O[:])

            for ci, (b0, b1) in enumerate(CHUNKS):
                h_chunk(b0, b1)
                if ci == 2:
                    # PE pstate ramp while the h chain races ahead; ends
                    # roughly when tile 0's FD matmuls become runnable
                    for _ in range(10):
                        nc.tensor.matmul(
                            jps[:], junk[:, :128], junk[:], start=True, stop=True
                        )
                for t in TILES_AFTER_CHUNK.get(ci, []):
                    do_tile(t)

    nc.finalize()
    return nc


_CACHE = {}


def _get_setup():
    if "nc" not in _CACHE:
        x, DX = _grid_x()
        _CACHE["x"] = x
        _CACHE["static"] = [_build_static(c, x, DX) for c in range(N_CORES)]
        _CACHE["nc"] = _build_program()
    return _CACHE["nc"], _CACHE["static"]


def _in_maps(BH_positions, BH_masses_presoftplus):
    nc, static = _get_setup()
    x = _CACHE["x"]
    pos = np.asarray(BH_positions, np.float64)
    pre = np.asarray(BH_masses_presoftplus, np.float64)
    mh = np.log1p(np.exp(pre)) * 0.5          # softplus(pre) / 2
    return [
        {
            "quad": _build_quad(static[c], x, pos, mh),
            "kv": static[c]["kv"],
            "dmat": static[c]["dmat"],
        }
        for c in range(N_CORES)
    ]


def kernel(BH_positions, BH_masses_presoftplus):
    from concourse.bass_utils import run_bass_kernel_spmd

    nc, _ = _get_setup()
    in_maps = _in_maps(BH_positions, BH_masses_presoftplus)
    res = run_bass_kernel_spmd(nc, in_maps, list(range(N_CORES)))
    parts = [
        res.results[c]["out"].reshape(PLANES, RES, RES, 3, 3, 3)
        for c in range(N_CORES)
    ]
    return np.ascontiguousarray(np.concatenate(parts, axis=0))
